# revision 18
# baseline (speedup 1.0000x reference)
"""AFNO2D Bass kernel for 8 TRN2 NeuronCores (v3).

Sharding: core k handles batch b=k//4 and channel group g=k%4, i.e. channels
[192*g, 192*(g+1)) = spectral-MLP blocks {2g, 2g+1}. The pipeline
(2D DHT -> block-diagonal MLP -> inverse DHT) is channel-local, no
collectives. The residual (+x) is added on the host after gathering.

All-bf16 design built on the CAS split  cas(a+b) = cas(a)cos(b) + cas(-a)sin(b):

  P1v: V[w,c,k1]   = sum_h U[h,w,c] CAS[h,k1]          (one product, not two)
  P2v: Xk[c,k1,k2] = sum_w V[w,c,k1] COS[w,k2]
                   + sum_w V[w,c,(128-k1)%128] SIN[w,k2]
       -> lands channel-major directly: no corner-turn transposes at all.
  P3:  f1 = relu(W1k^T Xk), ef = relu(W1nk^T Xk[flip])  (flip via reversed
                                                         moving reads)
  P4:  o2t[k2,k1,c] = f1^T W20 + ef^T (W20+W21)         (moving lhsT chunks)
  P5v: Wc[k1,c,w]  = sum_k2 o2t[k2,k1,c] CAS[k2,w]/16
  P6v: y[h,c,w]    = (sum_k1 COS[k1,h] Wc[k1,c,w]
                    + sum_k1 SIN[k1,h] Wc[k1,c,(128-w)%128]) / 1024
       w-flip handled by a 129-wide guard-column layout of Wc and a
       negative-stride moving read.

No fp8 anywhere (fp8-destination PSUM casts run at half rate on DVE/Act),
no DoubleRow, no PE transposes. PSUM evacuations alternate Scalar/Vector.

SBUF is one bf16 arena; regions are reused across stages (Tile interval
tracking orders the WAR/RAW hazards):
  R1 @ 0     (32768 el): xch [h,(w,c48)] -> f1|ef [o,pos] -> ybuf [h,(q,c4,w)]
  R2 @ 32768 (32768 el): xk [c96,(blk,pos)] -> wc [k1,(c,129)] guard layout
  R3 @ 65536 (24576 el): v [w,(c,k1)] -> o2t [k2,(k1,c)]
"""

import os
import numpy as np
import ml_dtypes

import concourse.bass as bass
import concourse.bacc as bacc
import concourse.mybir as mybir
import concourse.tile as tile
import concourse.bass_utils as bass_utils

BF16 = mybir.dt.bfloat16
F32 = mybir.dt.float32
COPY = mybir.ActivationFunctionType.Copy
RELU = mybir.ActivationFunctionType.Relu

H = 128
W = 128
NPOS = H * W          # 16384
CG = 192              # channels per core
BS = 96               # MLP block size
NCORES = 8

# arena element offsets (bf16 elems per partition); total 90112 el = 180224 B
E_XCH = 0             # x [h,(chunk4,w,c48)], 24576 el     (P1v in)
E_F1 = 0              # f1 [o96,pos], 16384 el             (P3 out)
E_EF = 16384          # ef [o96,pos], 16384 el
E_YBUF = 0            # ybuf [h,(q,c4,w)], 24576 el        (P6v out)
E_XK = 32768          # xk [c96,(blk,pos)], 32768 el       (P2v out / P3 in)
E_WC = 32768          # wc [k1,(c192,129)], 24768 el       (P5v out)
E_V = 65536           # v [w,(c192,k1)], 24576 el          (P1v out)
E_O2T = 65536         # o2t [k2,(k1,c192)], 24576 el       (P4 out)
AELEM = 90112


def _build_graph():
    nc = bacc.Bacc("TRN2", target_bir_lowering=False, debug=False)

    debug = bool(int(os.environ.get("AFNO_DEBUG", "0")))
    xq_e = [nc.declare_dram_parameter(f"xq{i}", [NPOS, 48], BF16, isOutput=False)
            for i in range(4)]
    dbg = {}
    if debug:
        for name, shape, dt in [
                ("dbg_v", [128, 24576], BF16),
                ("dbg_xk", [BS, 32768], BF16),
                ("dbg_f1", [BS, 2 * NPOS], BF16),
                ("dbg_ef", [BS, 2 * NPOS], BF16),
                ("dbg_o2t", [128, 24576], BF16),
                ("dbg_wc", [128, 24768], BF16)]:
            dbg[name] = nc.declare_dram_parameter(name, shape, dt, isOutput=True)
    cbf_e = nc.declare_dram_parameter("cbf", [128, 512], BF16, isOutput=False)
    wts_e = nc.declare_dram_parameter("wts", [BS, 8 * BS], BF16, isOutput=False)
    out_e = nc.declare_dram_parameter("out", [128, 24576], BF16, isOutput=True)

    with tile.TileContext(nc) as tc:
        with tc.tile_pool(name="sb", bufs=1) as sb:
            cbf = sb.tile([128, 512], BF16)
            wts = sb.tile([BS, 8 * BS], BF16)
            arena = sb.tile([128, AELEM], BF16)

            nc.sync.dma_start(cbf[:], cbf_e[:])
            nc.sync.dma_start(wts[:], wts_e[:])
            CAS = cbf[:, 0:128]               # cas(2pi jk/128)       (P1v rhs)
            CO = cbf[:, 128:256]              # cos                   (P2v/P6v)
            SI = cbf[:, 256:384]              # sin                   (P2v/P6v)
            CASI = cbf[:, 384:512]            # cas/16                (P5v rhs)

            def aview(off, nelem, rows=128):
                return arena[0:rows, off:off + nelem]

            def wslice(blk, j):  # j: 0=W1k 1=W1nk 2=W20 3=W20+W21
                o = (blk * 4 + j) * BS
                return wts[:, o:o + BS]

            def evac(i, dst, src, func=COPY, scale=None):
                if i % 2 == 0:
                    if scale is not None:
                        nc.scalar.activation(dst, src, func, scale=scale)
                    else:
                        nc.scalar.activation(dst, src, func)
                else:
                    if func == RELU:
                        nc.vector.tensor_scalar_max(dst, src, 0.0)
                    elif scale is not None:
                        nc.vector.tensor_scalar_mul(dst, src, scale)
                    else:
                        nc.vector.tensor_copy(dst, src)

            # x input: 4 chunks of 48 channels, layout [h, (c48, w)]
            # (host pre-transposes so each channel image column is contiguous)
            xch = [aview(E_XCH + 6144 * i, 6144) for i in range(4)]
            for i in range(4):
                src = xq_e[i].rearrange("(p a) b -> p (a b)", p=128)
                for s4 in range(4):
                    nc.sync.dma_start(xch[i][:, s4 * 1536:(s4 + 1) * 1536],
                                      src[:, s4 * 1536:(s4 + 1) * 1536])

            v = aview(E_V, 24576)
            vv = v.rearrange("p (k c) -> p k c", c=CG)   # k1-major

            pp_cm = tc.tile_pool(name="pp", bufs=8, space="PSUM")
            pp = pp_cm.__enter__()
            # PE warmup: ~40 back-to-back matmuls with no consumers ramp
            # the PE p-state to max while the x DMAs land.
            wt = pp.tile([128, 512], F32, tag="ps", name="ps_warm")
            for i in range(24):
                if i % 8 == 0:
                    wt = pp.tile([128, 512], F32, tag="ps", name="ps_warm")
                nc.tensor.matmul(wt[:, (i % 4) * 128:(i % 4) * 128 + 128],
                                 CAS, CAS, start=True, stop=True)

            # ---- P1v + P2v interleaved by half/block ----
            # P1v: V = U^T CAS -> v [w, (k1, c)]
            # P2v: Xk = V@CO + Vflip@SI -> xk [c96, (blk, pos)]
            xk = aview(E_XK, 32768, rows=BS)
            for blk in range(2):
                for i in range(24 * blk, 24 * blk + 24):
                    ps = pp.tile([128, 512], F32, tag="ps", name="ps_p1")
                    for u in range(4):
                        c = 4 * i + u
                        lhsT = xch[c // 48][:, (c % 48) * 128:(c % 48) * 128 + 128]
                        nc.tensor.matmul(ps[:, u * 128:(u + 1) * 128],
                                         lhsT, CAS, start=True, stop=True)
                    evac(i, vv[:, :, 4 * i:4 * i + 4],
                         ps.rearrange("p (c k) -> p k c", c=4))
                for g in range(32):
                    ps_ = pp.tile([128, 512], F32, tag="ps", name="ps_p2")
                    ps = ps_[0:BS, :]
                    for j in range(4):
                        k1 = 4 * g + j
                        k1f = (128 - k1) % 128
                        sl = ps[:, j * 128:(j + 1) * 128]
                        nc.tensor.matmul(
                            sl, vv[:, k1, blk * BS:(blk + 1) * BS], CO,
                            start=True, stop=False)
                        nc.tensor.matmul(
                            sl, vv[:, k1f, blk * BS:(blk + 1) * BS], SI,
                            start=False, stop=True)
                    o = blk * NPOS + g * 512
                    evac(g, xk[:, o:o + 512], ps[:])

            if debug:
                nc.sync.dma_start(dbg["dbg_v"][:], v)
                nc.sync.dma_start(dbg["dbg_xk"][:], xk)

            # ---- per block: P3 (MLP in), P4 (MLP out), then P5v half ----
            f1 = aview(E_F1, NPOS, rows=BS)
            ef = aview(E_EF, NPOS, rows=BS)
            o2t = aview(E_O2T, 24576)
            o2tv = o2t.rearrange("p (c k) -> p c k", c=CG)   # c-major
            wc = aview(E_WC, 24768)
            wcv = wc.rearrange("p (c j) -> p c j", c=CG)
            ybuf = aview(E_YBUF, 24576)

            if True:
                for blk in range(2):
                    xkb = xk[:, blk * NPOS:(blk + 1) * NPOS]
                    # P3: f1 = relu(W1k^T xk); ef = flip-read relu(W1nk^T xk)
                    for br in range(2):
                        for ch in range(32):
                            ps_ = pp.tile([128, 512], F32, tag="ps",
                                          name="ps_p3")
                            ps = ps_[0:BS, :]
                            if br == 0:
                                nc.tensor.matmul(
                                    ps[:], wslice(blk, 0),
                                    xkb[:, ch * 512:(ch + 1) * 512],
                                    start=True, stop=True)
                            elif ch == 0:
                                nc.tensor.matmul(ps[:, 0:1], wslice(blk, 1),
                                                 xkb[:, 0:1],
                                                 start=True, stop=True)
                                nc.tensor.matmul(ps[:, 1:512], wslice(blk, 1),
                                                 xkb[:, NPOS - 1:NPOS - 512:-1],
                                                 start=True, stop=True)
                            else:
                                e = NPOS - 512 * ch
                                nc.tensor.matmul(ps[:], wslice(blk, 1),
                                                 xkb[:, e:e - 512:-1],
                                                 start=True, stop=True)
                            dstb = f1 if br == 0 else ef
                            evac(ch + br, dstb[:, ch * 512:(ch + 1) * 512],
                                 ps[:], func=RELU)
                    if debug:
                        nc.sync.dma_start(
                            dbg["dbg_f1"][:, blk * NPOS:(blk + 1) * NPOS], f1)
                        nc.sync.dma_start(
                            dbg["dbg_ef"][:, blk * NPOS:(blk + 1) * NPOS], ef)
                    # P4: o2 = f1@W20 + ef@(W20+W21) -> o2t [k2,(k1,c192)]
                    for g in range(32):      # groups of 4 k1
                        ps_ = pp.tile([128, 512], F32, tag="ps", name="ps_p4")
                        ps = ps_[:, 0:384]
                        for j in range(4):
                            k1 = 4 * g + j
                            sl = ps[:, j * BS:(j + 1) * BS]
                            nc.tensor.matmul(sl,
                                             f1[:, k1 * 128:(k1 + 1) * 128],
                                             wslice(blk, 2),
                                             start=True, stop=False)
                            nc.tensor.matmul(sl,
                                             ef[:, k1 * 128:(k1 + 1) * 128],
                                             wslice(blk, 3),
                                             start=False, stop=True)
                        src = ps.rearrange("p (k c) -> p c k", k=4)
                        dst = o2tv[:, blk * BS:blk * BS + BS, 4 * g:4 * g + 4]
                        evac(g, dst, src)
                    # P5v (this block's channels):
                    # Wc[k1, c, w] = sum_k2 o2t[k2, k1, c] * CAS[k2, w]/16
                    for i in range(24):
                        ps = pp.tile([128, 512], F32, tag="ps", name="ps_p5")
                        for u in range(4):
                            c = blk * BS + 4 * i + u
                            nc.tensor.matmul(ps[:, u * 128:(u + 1) * 128],
                                             o2tv[:, c, :], CASI,
                                             start=True, stop=True)
                        c0 = blk * BS + 4 * i
                        evac(i, wcv[:, c0:c0 + 4, 0:128],
                             ps.rearrange("p (c w) -> p c w", c=4))
                    # guard column: wc[:, c, 128] = wc[:, c, 0]
                    nc.vector.tensor_copy(
                        wcv[:, blk * BS:(blk + 1) * BS, 128:129],
                        wcv[:, blk * BS:(blk + 1) * BS, 0:1])
                    # P6v: y = (CO^T Wc + SI^T Wc[flip w]) / 1024
                    for q in range(24 * blk, 24 * blk + 24):
                        ps = pp.tile([128, 512], F32, tag="ps", name="ps_p6")
                        c0 = 4 * q
                        nc.tensor.matmul(ps[:], CO, wcv[:, c0:c0 + 4, 0:128],
                                         start=True, stop=False)
                        nc.tensor.matmul(ps[:], SI, wcv[:, c0:c0 + 4, 128:0:-1],
                                         start=False, stop=True)
                        evac(q, ybuf[:, q * 512:(q + 1) * 512], ps[:],
                             scale=1.0 / 1024)
                        if q % 6 == 5:
                            r = q // 6
                            nc.sync.dma_start(
                                out_e[:, r * 3072:(r + 1) * 3072],
                                ybuf[:, r * 3072:(r + 1) * 3072])

            if debug:
                nc.sync.dma_start(dbg["dbg_o2t"][:], o2t)
                nc.sync.dma_start(dbg["dbg_wc"][:], wc)

            pp_cm.__exit__(None, None, None)

    nc.finalize()
    return nc


_NC_CACHE = None


def _get_graph():
    global _NC_CACHE
    if _NC_CACHE is None:
        _NC_CACHE = _build_graph()
    return _NC_CACHE


def _host_constants():
    k = np.arange(128)
    th = 2.0 * np.pi * np.outer(k, k) / 128.0
    A = np.cos(th)
    B = np.sin(th)
    cas = A + B
    cbf = np.concatenate([cas, A, B, cas / 16], axis=1)
    return cbf.astype(ml_dtypes.bfloat16)


def kernel(x, w1, w2):
    x = np.asarray(x, dtype=np.float32)
    w1 = np.asarray(w1, dtype=np.float32)
    w2 = np.asarray(w2, dtype=np.float32)
    assert x.shape == (2, NPOS, 768)

    cbf = _host_constants()
    xbf = x.astype(ml_dtypes.bfloat16)

    in_maps = []
    for core in range(NCORES):
        b, g = core // 4, core % 4
        m = {"cbf": cbf}
        for i in range(4):
            c0 = CG * g + 48 * i
            # [pos, 48] -> [h, c48, w] -> flattened [NPOS, 48] DMA payload
            xt = xbf[b, :, c0:c0 + 48].reshape(H, W, 48).transpose(0, 2, 1)
            m[f"xq{i}"] = np.ascontiguousarray(xt).reshape(NPOS, 48)
        wtsm = np.empty((BS, 8 * BS), np.float32)
        for blk2 in range(2):
            blk = 2 * g + blk2
            o = blk2 * 4 * BS
            wtsm[:, o + 0 * BS:o + 1 * BS] = w1[0, blk]
            wtsm[:, o + 1 * BS:o + 2 * BS] = w1[1, blk]
            wtsm[:, o + 2 * BS:o + 3 * BS] = w2[0, blk]
            wtsm[:, o + 3 * BS:o + 4 * BS] = w2[0, blk] + w2[1, blk]
        m["wts"] = wtsm.astype(ml_dtypes.bfloat16)
        in_maps.append(m)

    nc = _get_graph()
    trace = bool(int(os.environ.get("AFNO_TRACE", "0")))
    res = bass_utils.run_bass_kernel_spmd(
        nc, in_maps, list(range(NCORES)), trace=trace)
    kernel.last_result = res

    y = np.empty((2, NPOS, 768), np.float32)
    for core in range(NCORES):
        b, g = core // 4, core % 4
        o = res.results[core]["out"]          # [h, (q, c4, w)]
        yc = o.reshape(H, 48, 4, W).transpose(0, 3, 1, 2).reshape(NPOS, CG)
        y[b, :, CG * g:CG * (g + 1)] = yc.astype(np.float32)
    y += x
    return y
